# revision 2
# baseline (speedup 1.0000x reference)
# Trainium2 Bass kernel for nn_DiversityLoss (segment_reduce).
#
# Final version (v11 of the optimization session; baseline was 433us).
#
# Design history: v2 sorted-by-label one-hot-weight matmuls (79.5us);
# v3 LDWEIGHTS dedup + warmup (80.8); v4 DoubleRow fp8 + no-buffer-reuse
# (67.1, DMA-bound at the 358GB/s per-core HBM wall); v5-v7 chased the
# tail and found the PE runs at the cold 1.2GHz HAM clock for the first
# ~35us of every run (un-throttle not reliably controllable), making the
# cold PE (256 B/ns) the binding resource below the DMA rate; v8 tried
# DoubleRow + column tiling (walrus rejects: XBUS budget).
#
# v9: plain fp8 matmuls with 4-way PE COLUMN TILING. Slots map to array
# column-groups (slot s -> group s%4, index s//4), each group accumulates
# into its own PSUM bank at partitions 32g:32g+32, and the 4 streams are
# emitted interleaved so they execute CONCURRENTLY in the 16 32x32
# sub-arrays. Aggregate PE rate ~4 tiles/107ns cold (~612 B/ns) — above
# the DMA wall even when cold, so the kernel is DMA-bound throughout:
#   exec ~= head (~7us) + 16.8MB @ ~358GB/s (~48us) + tail (~3us).
#
# Correctness contract: host sorts rows by label (layout permutation),
# pads each class to whole 128-row tiles, device computes ALL per-class
# sums via PE matmuls (fp32 PSUM accumulate), host does counts (bincount)
# and the final means/variance in float64, exactly like the v1 baseline.
# fp8e4m3 embedding quantization gives rel err ~1e-3 (tol 2e-2).

import numpy as np
import ml_dtypes

N = 1_000_000
D = 128
C = 1000
CORES = 8
SLOTS = (C + CORES - 1) // CORES  # 125 class-slots per core
NGRP = 4  # PE column groups
GSLOT = 32  # slot indices per group
TPR = 128  # rows per tile
NDUMMY = 32
WTOT = 4 * 128  # four 128-col sliding one-hot variants

F8 = ml_dtypes.float8_e4m3

TRACE = False
TRACE_KWARGS = {}
LAST_RESULT = None

_cached = {}


def _dedup_ldweights(nc, mybir):
    """Drop InstLdweights whose weights AP equals the previous load's for
    the same array tile_position (weights persist per column-group)."""
    removed = 0
    for f in nc.m.functions:
        for bb in f.blocks:
            new_insts = []
            last_sig = {}
            for inst in bb.instructions:
                tn = type(inst).__name__
                if tn == "InstLdweights":
                    key = str(inst.tile_position)
                    sig = (
                        str(inst.ins[0]),
                        str(inst.perf_mode),
                        str(inst.is_transpose),
                    )
                    si = inst.sync_info
                    clean = si is None or (
                        len(si.on_wait) == 0 and len(si.on_update) == 0
                    )
                    if clean and last_sig.get(key) == sig:
                        removed += 1
                        continue
                    last_sig[key] = sig
                elif tn in ("InstMatmult", "InstEventSemaphore", "InstDrain"):
                    pass
                else:
                    if getattr(inst, "engine", None) == mybir.EngineType.PE:
                        last_sig = {}
                new_insts.append(inst)
            bb.instructions[:] = new_insts
    return removed


def _mm_schedule(T):
    """-> list of (tile_start, slot) per-tile MMs, interleaved across the
    4 column-groups (slot quads) so the array streams run concurrently."""
    starts = np.concatenate([[0], np.cumsum(T)])
    mms = []
    for p in range(0, len(T), NGRP):
        quad = [
            list(range(int(starts[s]), int(starts[s + 1])))
            for s in range(p, min(p + NGRP, len(T)))
        ]
        for i in range(max(len(q) for q in quad)):
            for gi, q in enumerate(quad):
                if i < len(q):
                    mms.append((q[i], p + gi))
    return mms, int(starts[len(T)])


def _chunk_splits(T_total):
    taper = [32, 16, 8]
    bulk_end = T_total - sum(taper)
    targets = [16, 48]
    g = 64
    while targets[-1] + g < bulk_end:
        targets.append(targets[-1] + g)
    targets.append(bulk_end)
    for w in taper:
        targets.append(targets[-1] + w)
    splits = [0]
    for t in targets:
        if t > splits[-1]:
            splits.append(min(t, T_total))
        if splits[-1] >= T_total:
            break
    if splits[-1] != T_total:
        splits.append(T_total)
    return splits


def _build_module(T):
    import bisect

    import concourse.mybir as mybir
    import concourse.tile as tile
    from concourse import bacc

    f8 = mybir.dt.float8e4
    f32 = mybir.dt.float32

    mms, T_total = _mm_schedule(T)
    splits = _chunk_splits(T_total)

    grp_mms = [[i for i, m in enumerate(mms) if m[1] % NGRP == g] for g in range(NGRP)]
    first_mm = [g[0] if g else -1 for g in grp_mms]
    last_mm = [g[-1] if g else -1 for g in grp_mms]

    mm_chunk = []
    for t0, s in mms:
        ch = bisect.bisect_right(splits, t0) - 1
        assert t0 + 1 <= splits[ch + 1]
        mm_chunk.append(ch)

    sizes = [splits[c + 1] - splits[c] for c in range(len(splits) - 1)]

    def pclass(sz):
        return 0 if sz <= 32 else (1 if sz <= 64 else 2)

    pool_cap = [32, 64, max(sizes)]
    pool_n = [max(1, sum(1 for sz in sizes if pclass(sz) == p)) for p in range(3)]

    nc = bacc.Bacc(
        "TRN2",
        target_bir_lowering=False,
        debug=False,
        enable_asserts=False,
        num_devices=CORES,
    )
    emb_d = nc.dram_tensor("emb", [128, T_total * D], f8, kind="ExternalInput")
    wgt_d = nc.dram_tensor("wgt", [128, WTOT], f8, kind="ExternalInput")
    out_d = nc.dram_tensor("out", [128, D], f32, kind="ExternalOutput")

    with tile.TileContext(nc) as tc:
        with (
            tc.tile_pool(name="consts", bufs=1) as consts,
            tc.tile_pool(name="ep0", bufs=pool_n[0]) as ep0,
            tc.tile_pool(name="ep1", bufs=pool_n[1]) as ep1,
            tc.tile_pool(name="ep2", bufs=pool_n[2]) as ep2,
            tc.tile_pool(name="psum", bufs=1, space="PSUM") as psum,
            tc.tile_pool(name="outb", bufs=1) as outb,
        ):
            wgt_t = consts.tile([128, WTOT], f8)
            nc.sync.dma_start(out=wgt_t[:], in_=wgt_d[:])

            # one full PSUM bank per column-group; group g accumulates in
            # partitions 32g:32g+32 (matches tile_position[1])
            psg = [
                psum.tile([128, 512], f32, name=f"psg{g}")
                for g in range(NGRP)
            ]
            out_t = outb.tile([128, D], f32)

            pools = [ep0, ep1, ep2]
            ets = []
            for ch in range(len(splits) - 1):
                t0, t1 = splits[ch], splits[ch + 1]
                p = pclass(t1 - t0)
                et = pools[p].tile([128, pool_cap[p] * D], f8)
                nc.sync.dma_start(
                    out=et[:, 0 : (t1 - t0) * D],
                    in_=emb_d[:, t0 * D : t1 * D],
                )
                ets.append(et)

            for i, (tt, s) in enumerate(mms):
                ch = mm_chunk[i]
                g = s % NGRP
                j = s // NGRP  # one-hot column within the 32-wide window
                u, r = divmod(j, 4)
                w0 = r * 128 + 64 - 4 * u
                nc.tensor.matmul(
                    psg[g][32 * g : 32 * g + 32, 0:D],
                    lhsT=wgt_t[:, w0 : w0 + GSLOT],
                    rhs=ets[ch][:, (tt - splits[ch]) * D : (tt - splits[ch] + 1) * D],
                    start=(i == first_mm[g]),
                    stop=(i == last_mm[g]),
                    skip_group_check=True,
                    tile_position=(0, 32 * g),
                )
                # flush each finished group while the others still run
                # (slot 124 -> group 0 is always the last to finish);
                # scalar and vector engines split the copies
                if i == last_mm[1]:
                    nc.scalar.copy(
                        out=out_t[32:64, :], in_=psg[1][32:64, 0:D]
                    )
                if i == last_mm[2]:
                    nc.vector.tensor_copy(
                        out=out_t[64:96, :], in_=psg[2][64:96, 0:D]
                    )
                if i == last_mm[3]:
                    nc.scalar.copy(
                        out=out_t[96:128, :], in_=psg[3][96:128, 0:D]
                    )

            nc.scalar.dma_start(out=out_d[32:128, :], in_=out_t[32:128, :])
            nc.vector.tensor_copy(out=out_t[0:32, :], in_=psg[0][0:32, 0:D])
            nc.scalar.dma_start(out=out_d[0:32, :], in_=out_t[0:32, :])

    _dedup_ldweights(nc, mybir)
    nc.compile()
    return nc


def _schedule(counts):
    rank = np.argsort(-counts, kind="stable")
    T = np.zeros(SLOTS, np.int64)
    for s in range(SLOTS):
        grp = counts[rank[s * CORES : (s + 1) * CORES]]
        mx = int(grp.max()) if len(grp) else 0
        T[s] = max(1, -(-mx // TPR))
    tile_off = np.concatenate([[0], np.cumsum(T)])
    return rank, T, tile_off


def _prep_inputs(embeddings, labels, counts, rank, T, tile_off):
    T_total = int(T.sum())
    emb8 = embeddings.astype(F8)

    order = np.argsort(labels, kind="stable")
    cls_start = np.concatenate([[0], np.cumsum(counts)])

    # four 128-col variants; variant r has its ones-column at col 64+r so
    # the 32-wide window [64+r-j ... ] puts the one at position j with a
    # 4-byte-aligned start
    wgt = np.zeros((128, WTOT), dtype=F8)
    for r in range(4):
        wgt[:, r * 128 + 64 + r] = 1.0

    in_maps = []
    for k in range(CORES):
        buf = np.zeros((T_total * TPR, D), dtype=F8)
        for s in range(SLOTS):
            ridx = s * CORES + k
            if ridx >= C:
                continue
            c = rank[ridx]
            n = int(counts[c])
            src = order[cls_start[c] : cls_start[c] + n]
            dst0 = int(tile_off[s]) * TPR
            buf[dst0 : dst0 + n] = emb8[src]
        emb_t = np.ascontiguousarray(
            buf.reshape(T_total, TPR, D).transpose(1, 0, 2)
        ).reshape(128, T_total * D)
        in_maps.append({"emb": emb_t, "wgt": wgt})
    return in_maps


def kernel(embeddings, labels):
    global LAST_RESULT
    from concourse.bass_utils import run_bass_kernel_spmd

    embeddings = np.asarray(embeddings)
    labels64 = np.asarray(labels).astype(np.int64)

    counts = np.bincount(labels64, minlength=C)
    rank, T, tile_off = _schedule(counts)

    key = tuple(T.tolist())
    if key not in _cached:
        _cached.clear()
        _cached[key] = _build_module(T)
    nc = _cached[key]

    in_maps = _prep_inputs(embeddings, labels64, counts, rank, T, tile_off)
    res = run_bass_kernel_spmd(
        nc,
        in_maps,
        core_ids=list(range(CORES)),
        trace=TRACE,
        **TRACE_KWARGS,
    )
    LAST_RESULT = res

    sums = np.zeros((C, D), dtype=np.float64)
    for k in range(CORES):
        out_k = res.results[k]["out"].astype(np.float64)
        for s in range(SLOTS):
            ridx = s * CORES + k
            if ridx < C:
                row = 32 * (s % NGRP) + s // NGRP
                sums[rank[ridx]] = out_k[row]

    cnt = counts.astype(np.float64)
    means = sums / cnt[:, None]
    mu = means.mean(axis=0)
    var = ((means - mu) ** 2).sum(axis=0) / (C - 1)
    return np.float32(-var.mean())


# revision 3
# speedup vs baseline: 1.0876x; 1.0876x over previous
# Trainium2 Bass kernel for nn_DiversityLoss (segment_reduce).
#
# Final version (v11 of the optimization session; baseline was 433us).
#
# Design history: v2 sorted-by-label one-hot-weight matmuls (79.5us);
# v3 LDWEIGHTS dedup + warmup (80.8); v4 DoubleRow fp8 + no-buffer-reuse
# (67.1, DMA-bound at the 358GB/s per-core HBM wall); v5-v7 chased the
# tail and found the PE runs at the cold 1.2GHz HAM clock for the first
# ~35us of every run (un-throttle not reliably controllable), making the
# cold PE (256 B/ns) the binding resource below the DMA rate; v8 tried
# DoubleRow + column tiling (walrus rejects: XBUS budget).
#
# v9: plain fp8 matmuls with 4-way PE COLUMN TILING. Slots map to array
# column-groups (slot s -> group s%4, index s//4), each group accumulates
# into its own PSUM bank at partitions 32g:32g+32, and the 4 streams are
# emitted interleaved so they execute CONCURRENTLY in the 16 32x32
# sub-arrays. Aggregate PE rate ~4 tiles/107ns cold (~612 B/ns) — above
# the DMA wall even when cold, so the kernel is DMA-bound throughout:
#   exec ~= head (~7us) + 16.8MB @ ~358GB/s (~48us) + tail (~3us).
#
# Correctness contract: host sorts rows by label (layout permutation),
# pads each class to whole 128-row tiles, device computes ALL per-class
# sums via PE matmuls (fp32 PSUM accumulate), host does counts (bincount)
# and the final means/variance in float64, exactly like the v1 baseline.
# fp8e4m3 embedding quantization gives rel err ~1e-3 (tol 2e-2).

import numpy as np
import ml_dtypes

N = 1_000_000
D = 128
C = 1000
CORES = 8
SLOTS = (C + CORES - 1) // CORES  # 125 class-slots per core
NGRP = 4  # PE column groups
GSLOT = 32  # slot indices per group
TPR = 128  # rows per tile
NDUMMY = 32
WTOT = 4 * 128  # four 128-col sliding one-hot variants

F8 = ml_dtypes.float8_e4m3

TRACE = False
TRACE_KWARGS = {}
LAST_RESULT = None

_cached = {}


def _dedup_ldweights(nc, mybir):
    """Drop InstLdweights whose weights AP equals the previous load's for
    the same array tile_position (weights persist per column-group)."""
    removed = 0
    for f in nc.m.functions:
        for bb in f.blocks:
            new_insts = []
            last_sig = {}
            for inst in bb.instructions:
                tn = type(inst).__name__
                if tn == "InstLdweights":
                    key = str(inst.tile_position)
                    sig = (
                        str(inst.ins[0]),
                        str(inst.perf_mode),
                        str(inst.is_transpose),
                    )
                    si = inst.sync_info
                    clean = si is None or (
                        len(si.on_wait) == 0 and len(si.on_update) == 0
                    )
                    if clean and last_sig.get(key) == sig:
                        removed += 1
                        continue
                    last_sig[key] = sig
                elif tn in ("InstMatmult", "InstEventSemaphore", "InstDrain"):
                    pass
                else:
                    if getattr(inst, "engine", None) == mybir.EngineType.PE:
                        last_sig = {}
                new_insts.append(inst)
            bb.instructions[:] = new_insts
    return removed


def _mm_schedule(T):
    """-> list of (tile_start, slot) per-tile MMs, interleaved across the
    4 column-groups (slot quads) so the array streams run concurrently."""
    starts = np.concatenate([[0], np.cumsum(T)])
    mms = []
    for p in range(0, len(T), NGRP):
        quad = [
            list(range(int(starts[s]), int(starts[s + 1])))
            for s in range(p, min(p + NGRP, len(T)))
        ]
        for i in range(max(len(q) for q in quad)):
            for gi, q in enumerate(quad):
                if i < len(q):
                    mms.append((q[i], p + gi))
    return mms, int(starts[len(T)])


def _chunk_splits(T_total):
    taper = [32, 16, 8]
    bulk_end = T_total - sum(taper)
    targets = [16, 48]
    g = 64
    while targets[-1] + g < bulk_end:
        targets.append(targets[-1] + g)
    targets.append(bulk_end)
    for w in taper:
        targets.append(targets[-1] + w)
    splits = [0]
    for t in targets:
        if t > splits[-1]:
            splits.append(min(t, T_total))
        if splits[-1] >= T_total:
            break
    if splits[-1] != T_total:
        splits.append(T_total)
    return splits


def _build_module(T):
    import bisect

    import concourse.mybir as mybir
    import concourse.tile as tile
    from concourse import bacc

    f8 = mybir.dt.float8e4
    f32 = mybir.dt.float32

    mms, T_total = _mm_schedule(T)
    splits = _chunk_splits(T_total)

    grp_mms = [[i for i, m in enumerate(mms) if m[1] % NGRP == g] for g in range(NGRP)]
    first_mm = [g[0] if g else -1 for g in grp_mms]
    last_mm = [g[-1] if g else -1 for g in grp_mms]

    mm_chunk = []
    for t0, s in mms:
        ch = bisect.bisect_right(splits, t0) - 1
        assert t0 + 1 <= splits[ch + 1]
        mm_chunk.append(ch)

    sizes = [splits[c + 1] - splits[c] for c in range(len(splits) - 1)]

    def pclass(sz):
        return 0 if sz <= 32 else (1 if sz <= 64 else 2)

    pool_cap = [32, 64, max(sizes)]
    pool_n = [max(1, sum(1 for sz in sizes if pclass(sz) == p)) for p in range(3)]

    nc = bacc.Bacc(
        "TRN2",
        target_bir_lowering=False,
        debug=False,
        enable_asserts=False,
        num_devices=CORES,
    )
    emb_d = nc.dram_tensor("emb", [128, T_total * D], f8, kind="ExternalInput")
    wgt_d = nc.dram_tensor("wgt", [128, WTOT], f8, kind="ExternalInput")
    out_d = nc.dram_tensor("out", [128, D], f32, kind="ExternalOutput")

    with tile.TileContext(nc) as tc:
        with (
            tc.tile_pool(name="consts", bufs=1) as consts,
            tc.tile_pool(name="ep0", bufs=pool_n[0]) as ep0,
            tc.tile_pool(name="ep1", bufs=pool_n[1]) as ep1,
            tc.tile_pool(name="ep2", bufs=pool_n[2]) as ep2,
            tc.tile_pool(name="psum", bufs=1, space="PSUM") as psum,
            tc.tile_pool(name="outb", bufs=1) as outb,
        ):
            wgt_t = consts.tile([128, WTOT], f8)
            nc.sync.dma_start(out=wgt_t[:], in_=wgt_d[:])

            # one full PSUM bank per column-group; group g accumulates in
            # partitions 32g:32g+32 (matches tile_position[1])
            psg = [
                psum.tile([128, 512], f32, name=f"psg{g}")
                for g in range(NGRP)
            ]
            out_t = outb.tile([128, D], f32)

            pools = [ep0, ep1, ep2]
            ets = []
            for ch in range(len(splits) - 1):
                t0, t1 = splits[ch], splits[ch + 1]
                p = pclass(t1 - t0)
                et = pools[p].tile([128, pool_cap[p] * D], f8)
                nc.sync.dma_start(
                    out=et[:, 0 : (t1 - t0) * D],
                    in_=emb_d[:, t0 * D : t1 * D],
                )
                ets.append(et)

            for i, (tt, s) in enumerate(mms):
                ch = mm_chunk[i]
                g = s % NGRP
                j = s // NGRP  # one-hot column within the 32-wide window
                u, r = divmod(j, 4)
                w0 = r * 128 + 64 - 4 * u
                nc.tensor.matmul(
                    psg[g][32 * g : 32 * g + 32, 0:D],
                    lhsT=wgt_t[:, w0 : w0 + GSLOT],
                    rhs=ets[ch][:, (tt - splits[ch]) * D : (tt - splits[ch] + 1) * D],
                    start=(i == first_mm[g]),
                    stop=(i == last_mm[g]),
                    skip_group_check=True,
                    tile_position=(0, 32 * g),
                )
                # flush each finished group while the others still run
                # (slot 124 -> group 0 is always the last to finish);
                # scalar and vector engines split the copies
                if i == last_mm[1]:
                    nc.scalar.copy(
                        out=out_t[32:64, :], in_=psg[1][32:64, 0:D]
                    )
                if i == last_mm[2]:
                    nc.vector.tensor_copy(
                        out=out_t[64:96, :], in_=psg[2][64:96, 0:D]
                    )
                if i == last_mm[3]:
                    nc.scalar.copy(
                        out=out_t[96:128, :], in_=psg[3][96:128, 0:D]
                    )

            nc.scalar.dma_start(out=out_d[32:128, :], in_=out_t[32:128, :])
            nc.vector.tensor_copy(out=out_t[0:32, :], in_=psg[0][0:32, 0:D])
            nc.scalar.dma_start(out=out_d[0:32, :], in_=out_t[0:32, :])

    _dedup_ldweights(nc, mybir)
    nc.compile()
    return nc


def _schedule(counts):
    rank = np.argsort(-counts, kind="stable")
    T = np.zeros(SLOTS, np.int64)
    for s in range(SLOTS):
        grp = counts[rank[s * CORES : (s + 1) * CORES]]
        mx = int(grp.max()) if len(grp) else 0
        T[s] = max(1, -(-mx // TPR))
    tile_off = np.concatenate([[0], np.cumsum(T)])
    return rank, T, tile_off


def _prep_inputs(embeddings, labels, counts, rank, T, tile_off):
    T_total = int(T.sum())
    emb8 = embeddings.astype(F8)

    order = np.argsort(labels, kind="stable")
    cls_start = np.concatenate([[0], np.cumsum(counts)])

    # four 128-col variants; variant r has its ones-column at col 64+r so
    # the 32-wide window [64+r-j ... ] puts the one at position j with a
    # 4-byte-aligned start
    wgt = np.zeros((128, WTOT), dtype=F8)
    for r in range(4):
        wgt[:, r * 128 + 64 + r] = 1.0

    in_maps = []
    for k in range(CORES):
        buf = np.zeros((T_total * TPR, D), dtype=F8)
        for s in range(SLOTS):
            ridx = s * CORES + k
            if ridx >= C:
                continue
            c = rank[ridx]
            n = int(counts[c])
            src = order[cls_start[c] : cls_start[c] + n]
            dst0 = int(tile_off[s]) * TPR
            buf[dst0 : dst0 + n] = emb8[src]
        emb_t = np.ascontiguousarray(
            buf.reshape(T_total, TPR, D).transpose(1, 0, 2)
        ).reshape(128, T_total * D)
        in_maps.append({"emb": emb_t, "wgt": wgt})
    return in_maps


def kernel(embeddings, labels):
    global LAST_RESULT
    from concourse.bass_utils import run_bass_kernel_spmd

    embeddings = np.asarray(embeddings)
    labels64 = np.asarray(labels).astype(np.int64)

    counts = np.bincount(labels64, minlength=C)
    rank, T, tile_off = _schedule(counts)

    key = tuple(T.tolist())
    if key not in _cached:
        _cached.clear()
        _cached[key] = _build_module(T)
    nc = _cached[key]

    in_maps = _prep_inputs(embeddings, labels64, counts, rank, T, tile_off)
    try:
        res = run_bass_kernel_spmd(
            nc,
            in_maps,
            core_ids=list(range(CORES)),
            trace=TRACE,
            **TRACE_KWARGS,
        )
    except Exception:
        # transient NRT_EXEC_UNIT_UNRECOVERABLE device wedge observed
        # ~1/10 runs; a single retry has always cleared it
        import time

        time.sleep(2.0)
        res = run_bass_kernel_spmd(
            nc,
            in_maps,
            core_ids=list(range(CORES)),
            trace=TRACE,
            **TRACE_KWARGS,
        )
    LAST_RESULT = res

    sums = np.zeros((C, D), dtype=np.float64)
    for k in range(CORES):
        out_k = res.results[k]["out"].astype(np.float64)
        for s in range(SLOTS):
            ridx = s * CORES + k
            if ridx < C:
                row = 32 * (s % NGRP) + s // NGRP
                sums[rank[ridx]] = out_k[row]

    cnt = counts.astype(np.float64)
    means = sums / cnt[:, None]
    mu = means.mean(axis=0)
    var = ((means - mu) ** 2).sum(axis=0) / (C - 1)
    return np.float32(-var.mean())
